# revision 35
# baseline (speedup 1.0000x reference)
"""Single-layer transformer LM head kernel for 8 Trainium2 NeuronCores.

Model (B=2, T=2048, D=1024, V=32000):
    x = tok_emb[idx] + pos_emb
    x = x + 0.125 * causal_attn(x@Wq, x@Wk, x@Wv)
    x = x + gelu(x@W1 + b1)@W2 + b2
    out = x@Wout + bout

Sharding (one uniform SPMD program on 8 cores):
  - trunk token-parallel: core c owns 512 tokens (batch c//4, block c%4)
  - K/V for the whole batch-sequence are recomputed locally on every core
    from fp8 embeddings; the host feeds each core the full-batch fp8
    embeddings with the 512-token blocks ROTATED so the core's own block
    is always first; causality lives in a per-core additive mask input.
  - final-hidden AllGather (bf16) across all 8 cores, split in two halves
    so the first half overlaps the tail of the MLP down-projection
  - logits vocab-parallel: each core does all 4096 tokens x 4000 vocab

Precision plan (gate is 2e-2 rel err; this lands ~4e-3):
  - attention path entirely fp8e4m3 with DoubleRow matmuls (2x PE rate):
    x8 -> Q/K/V proj -> scores -> exp -> attn8 -> attn@V. Static scales,
    folded into the activation scale/psum-drain constants.
  - residual/MLP/logits in bf16 (1x PE rate, same as fp32r, half the DMA).
  - all PSUM accumulation fp32. b1/b2 applied on device; pos_emb add and
    bout add happen on the host (bout is +0 for this model's inputs).

Scheduling notes:
  - only SP and Act have hardware DGE on TRN2; every dma_start goes out on
    one of those two rings (GPSIMD software DGE is much slower).  SP ring:
    weight/embedding loads + all output stores.  Act ring: late loads
    (masks, x0b, W2 tiles, Wout tiles, gathered hidden) ordered so no
    collective-gated descriptor sits ahead of one that is needed sooner.
  - PSUM drains alternate DVE/Act (GPSIMD cannot read PSUM).
  - a long-lived "bridge" pool holds the first two blocks of the gathered
    hidden state + the first Wout tile + output staging, in SBUF space that
    does NOT overlap the trunk pools -- so the first 8 logits tiles can
    start while the second AllGather half and the trunk SBUF teardown are
    still in flight.
"""
import math
import numpy as np
import ml_dtypes
import concourse.bass as bass
import concourse.bacc as bacc
import concourse.tile as tile
from concourse import bass_utils, mybir

F32 = mybir.dt.float32
F32R = mybir.dt.float32r
BF16 = mybir.dt.bfloat16
F8 = mybir.dt.float8e4
AF = mybir.ActivationFunctionType
OP = mybir.AluOpType
DR = mybir.MatmulPerfMode.DoubleRow

N_CORES = 8
B, T, D, DH, V = 2, 2048, 1024, 4096, 32000
TB = T // 4            # 512 tokens per core
VS = V // N_CORES      # 4000 vocab cols per core
VT = VS // 8           # 500 per n-tile
KC = D // 128          # 8 contraction chunks of d_model
HC = DH // 128         # 32 chunks of d_hidden
NTK = T // 128         # 16 key chunks (whole batch-sequence)
XH_ELEMS = (KC // 2) * 128 * TB   # half of one core's final-hidden block

# fp8 static scales (inputs are fixed-distribution: sigma~0.02 weights,
# sigma~0.028 embeddings; absmax headroom ~4x under fp8e4m3 max 448)
SX = 512.0             # x8 = fp8(x * SX)
SW = 2048.0            # Wq/Wk/Wv fp8 scale
SQKV = 1024.0          # q8/k8/v8 scale
PS2Q = SQKV / (SX * SW)            # psum -> q8/k8/v8 drain scale (2^-10)
SA = 64.0              # attn8 scale
EXPSCALE = (1.0 / 32.0) / (SQKV * SQKV)   # softmax scale / (sq*sk)
EXPBIAS = math.log(SA)
RSCONST = 0.125 / SQKV             # residual_scale / sv, applied via 1/l
MASKVAL = -1.0e10

_STATE = {}
_NO_COLL = False   # timing/sim variant: skip collectives


def _scaled_copy(nc, eng, out_ap, in_ap, scale):
    """PSUM->SBUF drain with scale on the given engine (DVE or Act)."""
    if eng is nc.scalar:
        nc.scalar.activation(out_ap, in_ap, AF.Copy, scale=scale)
    elif scale == 1.0:
        eng.tensor_copy(out_ap, in_ap)
    else:
        eng.tensor_scalar_mul(out_ap, in_ap, scale)


def _trunk(nc, tc, io, br, bounce_x1, bounce_x2, ag_x1, ag_x2):
    """Token-parallel trunk; ends with the split final-hidden AllGather."""
    copy_engines = [nc.vector, nc.scalar]   # PSUM readers (GPSIMD cannot)

    with tc.tile_pool(name="trunk", bufs=1) as pp, \
         tc.tile_pool(name="kv", bufs=1) as kvp:
        x0b = pp.tile([128, KC, TB], BF16)
        x1T = pp.tile([128, KC, TB], BF16)
        rs_b = pp.tile([128, TB], F32)
        q8 = kvp.tile([128, KC, TB], F8)
        k8 = kvp.tile([128, KC, T], F8)
        v8 = kvp.tile([128, NTK, D], F8)
        msk_s = kvp.tile([128, NTK, TB], BF16)

        # ---- x8 arrives per 512-token block, pipelined with V proj ----
        with tc.tile_pool(name="xw", bufs=1) as xw, \
             tc.tile_pool(name="ps_kv", bufs=6, space="PSUM") as pskv:
            x8s = xw.tile([128, 4, KC, TB], F8)
            wv8 = xw.tile([128, KC, D], F8)
            wk8 = xw.tile([128, KC, KC, 128], F8)
            wq8 = xw.tile([128, KC, KC, 128], F8)
            for k2 in range(0, KC, 2):
                nc.sync.dma_start(wv8[:, k2:k2 + 2, :],
                                  io["wv8"].ap()[:, k2:k2 + 2, :])
                nc.sync.dma_start(x8s[:, 0, k2:k2 + 2, :],
                                  io["x8"].ap()[0][:, k2:k2 + 2, :])
            nci = 0
            for tb in range(4):
                if tb == 0:
                    pass
                else:
                    nc.sync.dma_start(x8s[:, tb], io["x8"].ap()[tb])
                if tb == 3:
                    nc.sync.dma_start(wk8[:], io["wk8"].ap())
                    nc.sync.dma_start(wq8[:], io["wq8"].ap())
                # V projection for this block: psum [128 tok, 512 dv]
                for tc_ in range(4 * tb, 4 * tb + 4):
                    for h in range(2):
                        ps = pskv.tile([128, 512], F32, name="ps_kv")
                        for k2 in range(0, KC, 2):
                            nc.tensor.matmul(
                                ps[:],
                                x8s[:, tb, k2:k2 + 2, bass.ts(tc_ % 4, 128)],
                                wv8[:, k2:k2 + 2, bass.ts(h, 512)],
                                start=(k2 == 0), stop=(k2 == KC - 2),
                                perf_mode=DR)
                        eng = copy_engines[nci % 2]; nci += 1
                        _scaled_copy(nc, eng, v8[:, tc_, bass.ts(h, 512)],
                                     ps[:], PS2Q)

            # ---- K projection (full rotated sequence) -> k8 ----
            nc.scalar.dma_start(msk_s[:], io["mask"].ap())
            nc.scalar.dma_start(x0b[:], io["x0b"].ap())
            nc.scalar.dma_start(br["wot0"][:], io["woutb"].ap()[:, 0])
            for tb in range(4):
                for m in range(KC):
                    ps = pskv.tile([128, TB], F32, name="ps_kv")
                    for k2 in range(0, KC, 2):
                        nc.tensor.matmul(
                            ps[:], wk8[:, k2:k2 + 2, m, :],
                            x8s[:, tb, k2:k2 + 2, :],
                            start=(k2 == 0), stop=(k2 == KC - 2),
                            perf_mode=DR)
                    eng = copy_engines[nci % 2]; nci += 1
                    _scaled_copy(nc, eng, k8[:, m, bass.ts(tb, TB)],
                                 ps[:], PS2Q)

            # ---- Q projection (own block = rotated block 0) -> q8 ----
            for m in range(KC):
                ps = pskv.tile([128, TB], F32, name="ps_kv")
                for k2 in range(0, KC, 2):
                    nc.tensor.matmul(
                        ps[:], wq8[:, k2:k2 + 2, m, :],
                        x8s[:, 0, k2:k2 + 2, :],
                        start=(k2 == 0), stop=(k2 == KC - 2), perf_mode=DR)
                eng = copy_engines[nci % 2]; nci += 1
                _scaled_copy(nc, eng, q8[:, m, :], ps[:], PS2Q)

        # ---------- attention (scores transposed: sT[tk, tq]) ----------
        with tc.tile_pool(name="attn", bufs=1) as ap_, \
             tc.tile_pool(name="stmp", bufs=6) as stp, \
             tc.tile_pool(name="ps_sc", bufs=4, space="PSUM") as ps_sc, \
             tc.tile_pool(name="ps_l", bufs=1, space="PSUM") as ps_lp:
            attn8 = ap_.tile([128, NTK, TB], F8)
            ones2 = ap_.tile([128, 2, 16], F8)   # 16B k-substep for dual-fp8
            nc.vector.memset(ones2[:], 1.0)
            ones_f = ap_.tile([1, 128], F32)
            nc.vector.memset(ones_f[:], 1.0)
            ones_r = ap_.tile([1, 128], F32R)
            nc.vector.tensor_copy(ones_r[:], ones_f[:])
            ebias = ap_.tile([128, 1], F32)
            nc.vector.memset(ebias[:], EXPBIAS)
            ps_l = ps_lp.tile([1, TB], F32)

            for tkc in range(NTK):
                ps = ps_sc.tile([128, TB], F32, name="ps_s")
                for k2 in range(0, KC, 2):
                    nc.tensor.matmul(
                        ps[:], k8[:, k2:k2 + 2, bass.ts(tkc, 128)],
                        q8[:, k2:k2 + 2, :],
                        start=(k2 == 0), stop=(k2 == KC - 2), perf_mode=DR)
                stmp = stp.tile([128, TB], BF16, name="stmp")
                nc.vector.tensor_tensor(out=stmp[:], in0=ps[:],
                                        in1=msk_s[:, tkc, :], op=OP.add)
                nc.scalar.activation(attn8[:, tkc, :], stmp[:], AF.Exp,
                                     bias=ebias[:], scale=EXPSCALE)
            # softmax normalizer: deferred ones-matmuls (2 chunks per mm)
            for tkc in range(1, NTK, 2):
                nc.tensor.matmul(ps_l[:], ones2[:, :, 0:1],
                                 attn8[:, tkc - 1:tkc + 1, :],
                                 start=(tkc == 1), stop=(tkc == NTK - 1),
                                 perf_mode=DR)

            # rs = 0.125/(sv*l) broadcast to all partitions via PE matmul
            rs_row = ap_.tile([1, TB], F32)
            nc.vector.reciprocal(rs_row[:], ps_l[:])
            rs_row2 = ap_.tile([1, TB], F32R)
            nc.vector.tensor_scalar_mul(rs_row2[:], rs_row[:], RSCONST)
            ps_b = ps_lp.tile([128, TB], F32, name="ps_b")
            nc.tensor.matmul(ps_b[:], ones_r[:], rs_row2[:],
                             start=True, stop=True)
            nc.vector.tensor_copy(rs_b[:], ps_b[:])

            # oT[dv, tq] = V.T @ attnT ; x1T = x0 + rs * oT
            for m in range(KC):
                ps = ps_sc.tile([128, TB], F32, name="ps_s")
                for t2 in range(0, NTK, 2):
                    nc.tensor.matmul(ps[:], v8[:, t2:t2 + 2, bass.ts(m, 128)],
                                     attn8[:, t2:t2 + 2, :],
                                     start=(t2 == 0), stop=(t2 == NTK - 2),
                                     perf_mode=DR)
                ot = stp.tile([128, TB], F32, name="otmp")
                nc.vector.tensor_tensor(out=ot[:], in0=ps[:], in1=rs_b[:],
                                        op=OP.mult)
                # adds alternate DVE/Pool so neither engine's serial chain
                # outlasts the AV matmul stream
                aeng = nc.vector if m % 2 else nc.gpsimd
                aeng.tensor_tensor(out=x1T[:, m, :], in0=ot[:],
                                   in1=x0b[:, m, :], op=OP.add)

        # ---------- MLP ----------
        with tc.tile_pool(name="mlp", bufs=1) as mp, \
             tc.tile_pool(name="w1p", bufs=6) as w1p, \
             tc.tile_pool(name="w2p", bufs=6) as w2p, \
             tc.tile_pool(name="ps_h", bufs=6, space="PSUM") as ps_hp:
            b1_s = mp.tile([128, HC], F32)
            b2_s = mp.tile([128, KC], F32)
            nc.sync.dma_start(b1_s[:], io["b1t"].ap())
            nc.sync.dma_start(b2_s[:], io["b2t"].ap())
            hT = mp.tile([128, HC, TB], BF16)
            for m in range(HC):
                w1t = w1p.tile([128, KC, 128], BF16, name="w1t")
                nc.sync.dma_start(w1t[:], io["w1b"].ap()[:, m])
                ps = ps_hp.tile([128, TB], F32, name="ps_mlp")
                for k in range(KC):
                    nc.tensor.matmul(ps[:], w1t[:, k, :], x1T[:, k, :],
                                     start=(k == 0), stop=(k == KC - 1))
                nc.scalar.activation(hT[:, m, :], ps[:], AF.Gelu,
                                     bias=b1_s[:, m:m + 1], scale=1.0)
            x2T = mp.tile([128, KC, TB], BF16)
            for m in range(KC):
                w2t = w2p.tile([128, HC, 128], BF16, name="w2t")
                nc.scalar.dma_start(w2t[:], io["w2b"].ap()[:, m])
                ps = ps_hp.tile([128, TB], F32, name="ps_mlp")
                for k in range(HC):
                    nc.tensor.matmul(ps[:], w2t[:, k, :], hT[:, k, :],
                                     start=(k == 0), stop=(k == HC - 1))
                # x2T = (psum + b2) + x1T
                nc.vector.scalar_tensor_tensor(
                    out=x2T[:, m, :], in0=ps[:], scalar=b2_s[:, m:m + 1],
                    in1=x1T[:, m, :], op0=OP.add, op1=OP.add)
                # split final-hidden AllGather: first half overlaps m=4..7
                if m == KC // 2 - 1:
                    nc.sync.dma_start(
                        bounce_x1[:].rearrange("(k p t) -> p k t",
                                               k=KC // 2, p=128),
                        x2T[:, :KC // 2, :])
                    if not _NO_COLL:
                        nc.gpsimd.collective_compute(
                            "AllGather", OP.bypass,
                            replica_groups=[list(range(N_CORES))],
                            ins=[bounce_x1.opt()], outs=[ag_x1.opt()])
            nc.sync.dma_start(
                bounce_x2[:].rearrange("(k p t) -> p k t", k=KC // 2, p=128),
                x2T[:, KC // 2:, :])
            if not _NO_COLL:
                nc.gpsimd.collective_compute(
                    "AllGather", OP.bypass,
                    replica_groups=[list(range(N_CORES))],
                    ins=[bounce_x2.opt()], outs=[ag_x2.opt()])
            # bridge hidden blocks (r=0,1): issued after every w2t load so a
            # collective-gated descriptor never blocks a sooner-needed one
            for r in range(2):
                nc.scalar.dma_start(
                    br["xf01"][:, KC * r:KC * r + KC // 2, :],
                    ag_x1[r].rearrange("(k p t) -> p k t",
                                       k=KC // 2, p=128))
            for r in range(2):
                nc.scalar.dma_start(
                    br["xf01"][:, KC * r + KC // 2:KC * (r + 1), :],
                    ag_x2[r].rearrange("(k p t) -> p k t",
                                       k=KC // 2, p=128))


def _logits(nc, tc, io, br, ag_x1, ag_x2):
    """Vocab-parallel logits over the AllGathered final hidden states."""
    out_d = io["logits"]
    copy_engines = [nc.vector, nc.scalar]
    xf01, wot0, outp = br["xf01"], br["wot0"], br["outp"]
    with tc.tile_pool(name="lgp", bufs=1) as lp, \
         tc.tile_pool(name="wop", bufs=2) as wop, \
         tc.tile_pool(name="ps_lg", bufs=8, space="PSUM") as ps_lg:
        # hidden blocks r=2..7 live in SBUF space recycled from the trunk
        xfr = lp.tile([128, (N_CORES - 2) * KC, TB], BF16)

        def xslice(r, k):
            if r < 2:
                return xf01[:, KC * r + k, :]
            return xfr[:, KC * (r - 2) + k, :]

        def load_xfr_half(r, half):
            ag = ag_x1 if half == 0 else ag_x2
            off = KC * (r - 2) + half * (KC // 2)
            nc.scalar.dma_start(
                xfr[:, off:off + KC // 2, :],
                ag[r].rearrange("(k p t) -> p k t", k=KC // 2, p=128))

        def mm_half(ps, r, t4, wot, half):
            for k in range(half * (KC // 2), (half + 1) * (KC // 2)):
                nc.tensor.matmul(
                    ps[:], xslice(r, k)[:, bass.ts(t4, 128)],
                    wot[:, k, :],
                    start=(k == 0), stop=(k == KC - 1))

        nci = 0

        def drain(ps, r, n, t4):
            nonlocal nci
            ot = outp.tile([128, VT], F32, name="og")
            eng = copy_engines[nci % 2]; nci += 1
            _scaled_copy(nc, eng, ot[:], ps[:], 1.0)
            nc.sync.dma_start(out_d.ap()[r, n, bass.ts(t4, 128), :], ot[:])

        # remaining hidden blocks (r=2..7): first-needed first
        for r in range(2, N_CORES):
            load_xfr_half(r, 0)
            load_xfr_half(r, 1)

        # bridge tiles: the first 8 output tiles start on the ag_x1 half of
        # the contraction so PE has work while ag_x2 is still in flight
        bridge = []
        for r in range(2):
            for t4 in range(4):
                ps = ps_lg.tile([128, VT], F32, name="ps_g")
                mm_half(ps, r, t4, wot0, 0)
                bridge.append((ps, r, t4))
        for ps, r, t4 in bridge:
            mm_half(ps, r, t4, wot0, 1)
            drain(ps, r, 0, t4)

        for n in range(8):
            if n == 0:
                wot = wot0
            else:
                wot = wop.tile([128, KC, VT], BF16, name="wot")
                nc.scalar.dma_start(wot[:], io["woutb"].ap()[:, n])
            for r in range(N_CORES):
                if n == 0 and r < 2:
                    continue   # bridge tiles already done
                for t4 in range(4):
                    ps = ps_lg.tile([128, VT], F32, name="ps_g")
                    mm_half(ps, r, t4, wot, 0)
                    mm_half(ps, r, t4, wot, 1)
                    drain(ps, r, n, t4)


def _build(repeat=1, phases="full"):
    nc = bacc.Bacc("TRN2", target_bir_lowering=False, debug=False,
                   num_devices=N_CORES)

    # ---- kernel I/O (per-core shards prepared on host) ----
    io = {}
    def inp(name, shape, dt=F32):
        io[name] = nc.dram_tensor(name, shape, dt, kind="ExternalInput")
    inp("x8", [4, 128, KC, TB], F8)
    inp("x0b", [128, KC, TB], BF16)
    inp("wq8", [128, KC, KC, 128], F8)
    inp("wk8", [128, KC, KC, 128], F8)
    inp("wv8", [128, KC, D], F8)
    inp("w1b", [128, HC, KC, 128], BF16)
    inp("b1t", [128, HC])
    inp("w2b", [128, KC, HC, 128], BF16)
    inp("b2t", [128, KC])
    inp("woutb", [128, 8, KC, VT], BF16)
    inp("mask", [128, NTK, TB], BF16)
    io["logits"] = nc.dram_tensor("logits", [N_CORES, 8, TB, VT], F32,
                                  kind="ExternalOutput")

    with tile.TileContext(nc) as tc:
        with tc.tile_pool(name="dram", bufs=1, space="DRAM") as dp:
            for _ in range(repeat):  # repeat>1 is a timing-only variant
                bounce_x1 = dp.tile([XH_ELEMS], BF16, name="bounce_x1")
                bounce_x2 = dp.tile([XH_ELEMS], BF16, name="bounce_x2")
                ag_x1 = dp.tile([N_CORES, XH_ELEMS], BF16, name="ag_x1",
                                addr_space="Shared")
                ag_x2 = dp.tile([N_CORES, XH_ELEMS], BF16, name="ag_x2",
                                addr_space="Shared")
                with tc.tile_pool(name="bridge", bufs=1) as bp, \
                     tc.tile_pool(name="outp", bufs=6) as outp:
                    br = {
                        "xf01": bp.tile([128, 2 * KC, TB], BF16,
                                        name="xf01"),
                        "wot0": bp.tile([128, KC, VT], BF16, name="wot0"),
                        "outp": outp,
                    }
                    if phases in ("full", "trunk"):
                        _trunk(nc, tc, io, br, bounce_x1, bounce_x2,
                               ag_x1, ag_x2)
                    if phases in ("full", "logits"):
                        _logits(nc, tc, io, br, ag_x1, ag_x2)

    nc.compile()
    return nc


F8NP = ml_dtypes.float8_e4m3fn
BFNP = ml_dtypes.bfloat16


def _prep_shared(Wq, Wk, Wv, W1, b1, W2, b2, pos_emb):
    f = np.float32
    sh = {}
    sh["wq8"] = np.ascontiguousarray(
        (Wq * SW).reshape(KC, 128, KC, 128).transpose(1, 0, 2, 3)
    ).astype(F8NP)
    sh["wk8"] = np.ascontiguousarray(
        (Wk * SW).reshape(KC, 128, KC, 128).transpose(1, 0, 2, 3)
    ).astype(F8NP)
    sh["wv8"] = np.ascontiguousarray(
        (Wv * SW).reshape(KC, 128, D).transpose(1, 0, 2)).astype(F8NP)
    sh["w1b"] = np.ascontiguousarray(
        W1.reshape(KC, 128, HC, 128).transpose(1, 2, 0, 3)).astype(BFNP)
    sh["b1t"] = np.ascontiguousarray(b1.reshape(HC, 128).T, dtype=f)
    sh["w2b"] = np.ascontiguousarray(
        W2.reshape(HC, 128, KC, 128).transpose(1, 2, 0, 3)).astype(BFNP)
    sh["b2t"] = np.ascontiguousarray(b2.reshape(KC, 128).T, dtype=f)

    # per-j rotated block order and causal masks.
    # rotation: the core owning block j sees blocks in order [j, j+1, j+2,
    # j+3] (mod 4), so its own 512 tokens are always columns 0:TB.
    orders = [[(j + i) % 4 for i in range(4)] for j in range(4)]
    masks = []
    rr = np.arange(128)[:, None]
    cc = np.arange(TB)[None, :]
    for j in range(4):
        m = np.empty((NTK, 128, TB), dtype=f)
        for tkc in range(NTK):
            gtk = TB * orders[j][tkc // 4] + 128 * (tkc % 4) + rr
            m[tkc] = np.where(gtk <= TB * j + cc, 0.0, MASKVAL)
        masks.append(
            np.ascontiguousarray(m.transpose(1, 0, 2)).astype(BFNP))
    return sh, orders, masks


def make_in_maps(idx, tok_emb, pos_emb, Wq, Wk, Wv, W1, b1, W2, b2,
                 Wout, bout):
    f = np.float32
    tok_emb = np.asarray(tok_emb, dtype=f)
    pos = np.asarray(pos_emb, dtype=f)[:T]
    idx = np.asarray(idx)
    sh, orders, masks = _prep_shared(
        np.asarray(Wq, f), np.asarray(Wk, f), np.asarray(Wv, f),
        np.asarray(W1, f), np.asarray(b1, f), np.asarray(W2, f),
        np.asarray(b2, f), pos)
    Wout = np.asarray(Wout, f)

    x_full = [tok_emb[np.asarray(idx[b], dtype=np.int64)] + pos
              for b in range(B)]
    in_maps = []
    for c in range(N_CORES):
        b, j = c // 4, c % 4
        xr = np.concatenate([x_full[b][TB * br:TB * (br + 1)]
                             for br in orders[j]])           # [T, D] rotated
        x8pm = (xr.T * SX).reshape(KC, 128, T).transpose(1, 0, 2)  # [128,KC,T]
        m = dict(sh)
        m["x8"] = np.ascontiguousarray(
            x8pm.reshape(128, KC, 4, TB).transpose(2, 0, 1, 3)).astype(F8NP)
        m["x0b"] = np.ascontiguousarray(
            x_full[b][TB * j:TB * (j + 1)].T.reshape(KC, 128, TB)
            .transpose(1, 0, 2)).astype(BFNP)
        m["mask"] = masks[j]
        ws = Wout[:, VS * c:VS * (c + 1)]
        m["woutb"] = np.ascontiguousarray(
            ws.reshape(KC, 128, 8, VT).transpose(1, 2, 0, 3)).astype(BFNP)
        in_maps.append(m)
    return in_maps


def kernel(idx, tok_emb, pos_emb, Wq, Wk, Wv, W1, b1, W2, b2, Wout, bout):
    if "nc" not in _STATE:
        _STATE["nc"] = _build()
    nc = _STATE["nc"]

    in_maps = make_in_maps(idx, tok_emb, pos_emb, Wq, Wk, Wv, W1, b1, W2,
                           b2, Wout, bout)
    res = bass_utils.run_bass_kernel_spmd(nc, in_maps,
                                          core_ids=list(range(N_CORES)))
    _STATE["last_results"] = res

    out = np.empty((B * T, V), dtype=np.float32)
    for c in range(N_CORES):
        lg = res.results[c]["logits"]             # [8, 8, 512, 500]
        out[:, VS * c:VS * (c + 1)] = (
            lg.transpose(0, 2, 1, 3).reshape(B * T, VS))
    out += np.asarray(bout, np.float32)[None, :]
    return out.reshape(B, T, V)


# revision 36
# speedup vs baseline: 1.0908x; 1.0908x over previous
"""Single-layer transformer LM head kernel for 8 Trainium2 NeuronCores.

Model (B=2, T=2048, D=1024, V=32000):
    x = tok_emb[idx] + pos_emb
    x = x + 0.125 * causal_attn(x@Wq, x@Wk, x@Wv)
    x = x + gelu(x@W1 + b1)@W2 + b2
    out = x@Wout + bout

Sharding (one uniform SPMD program on 8 cores):
  - trunk token-parallel: core c owns 512 tokens (batch c//4, block c%4)
  - K/V for the whole batch-sequence are recomputed locally on every core
    from fp8 embeddings; the host feeds each core the full-batch fp8
    embeddings with the 512-token blocks ROTATED so the core's own block
    is always first; causality lives in a per-core additive mask input.
  - final-hidden AllGather (bf16) across all 8 cores, split in two halves
    so the first half overlaps the tail of the MLP down-projection
  - logits vocab-parallel: each core does all 4096 tokens x 4000 vocab

Precision plan (gate is 2e-2 rel err; this lands ~4e-3):
  - attention path entirely fp8e4m3 with DoubleRow matmuls (2x PE rate):
    x8 -> Q/K/V proj -> scores -> exp -> attn8 -> attn@V. Static scales,
    folded into the activation scale/psum-drain constants.
  - residual/MLP/logits in bf16 (1x PE rate, same as fp32r, half the DMA).
  - all PSUM accumulation fp32. b1/b2 applied on device; pos_emb add and
    bout add happen on the host (bout is +0 for this model's inputs).

Scheduling notes:
  - only SP and Act have hardware DGE on TRN2; every dma_start goes out on
    one of those two rings (GPSIMD software DGE is much slower).  SP ring:
    weight/embedding loads + all output stores.  Act ring: late loads
    (masks, x0b, W2 tiles, Wout tiles, gathered hidden) ordered so no
    collective-gated descriptor sits ahead of one that is needed sooner.
  - PSUM drains alternate DVE/Act (GPSIMD cannot read PSUM).
  - a long-lived "bridge" pool holds the first two blocks of the gathered
    hidden state + the first Wout tile + output staging, in SBUF space that
    does NOT overlap the trunk pools -- so the first 8 logits tiles can
    start while the second AllGather half and the trunk SBUF teardown are
    still in flight.
"""
import math
import numpy as np
import ml_dtypes
import concourse.bass as bass
import concourse.bacc as bacc
import concourse.tile as tile
from concourse import bass_utils, mybir

F32 = mybir.dt.float32
F32R = mybir.dt.float32r
BF16 = mybir.dt.bfloat16
F8 = mybir.dt.float8e4
AF = mybir.ActivationFunctionType
OP = mybir.AluOpType
DR = mybir.MatmulPerfMode.DoubleRow

N_CORES = 8
B, T, D, DH, V = 2, 2048, 1024, 4096, 32000
TB = T // 4            # 512 tokens per core
VS = V // N_CORES      # 4000 vocab cols per core
VT = VS // 8           # 500 per n-tile
KC = D // 128          # 8 contraction chunks of d_model
HC = DH // 128         # 32 chunks of d_hidden
NTK = T // 128         # 16 key chunks (whole batch-sequence)
XH_ELEMS = (KC // 2) * 128 * TB   # half of one core's final-hidden block

# fp8 static scales (inputs are fixed-distribution: sigma~0.02 weights,
# sigma~0.028 embeddings; absmax headroom ~4x under fp8e4m3 max 448)
SX = 512.0             # x8 = fp8(x * SX)
SW = 2048.0            # Wq/Wk/Wv fp8 scale
SQKV = 1024.0          # q8/k8/v8 scale
PS2Q = SQKV / (SX * SW)            # psum -> q8/k8/v8 drain scale (2^-10)
SA = 64.0              # attn8 scale
EXPSCALE = (1.0 / 32.0) / (SQKV * SQKV)   # softmax scale / (sq*sk)
EXPBIAS = math.log(SA)
RSCONST = 0.125 / SQKV             # residual_scale / sv, applied via 1/l
SX1 = 512.0            # x1 fp8 scale (W1 GEMM runs fp8 DoubleRow)
SW1 = 2048.0           # W1 fp8 scale
G1SCALE = 1.0 / (SX1 * SW1)        # gelu input scale (2^-20, exact)
MASKVAL = -1.0e10

_STATE = {}
_NO_COLL = False   # timing/sim variant: skip collectives


def _scaled_copy(nc, eng, out_ap, in_ap, scale):
    """PSUM->SBUF drain with scale on the given engine (DVE or Act)."""
    if eng is nc.scalar:
        nc.scalar.activation(out_ap, in_ap, AF.Copy, scale=scale)
    elif scale == 1.0:
        eng.tensor_copy(out_ap, in_ap)
    else:
        eng.tensor_scalar_mul(out_ap, in_ap, scale)


def _trunk(nc, tc, io, br, bounce_x1, bounce_x2, ag_x1, ag_x2):
    """Token-parallel trunk; ends with the split final-hidden AllGather."""
    copy_engines = [nc.vector, nc.scalar]   # PSUM readers (GPSIMD cannot)

    with tc.tile_pool(name="trunk", bufs=1) as pp, \
         tc.tile_pool(name="kv", bufs=1) as kvp:
        x0b = pp.tile([128, KC, TB], BF16)
        x1T = pp.tile([128, KC, TB], BF16)
        x18 = pp.tile([128, KC, TB], F8)
        rs_b = pp.tile([128, TB], F32)
        q8 = kvp.tile([128, KC, TB], F8)
        k8 = kvp.tile([128, KC, T], F8)
        v8 = kvp.tile([128, NTK, D], F8)
        msk_s = kvp.tile([128, NTK, TB], BF16)

        # ---- x8 arrives per 512-token block, pipelined with V proj ----
        with tc.tile_pool(name="xw", bufs=1) as xw, \
             tc.tile_pool(name="ps_kv", bufs=6, space="PSUM") as pskv:
            x8s = xw.tile([128, 4, KC, TB], F8)
            wv8 = xw.tile([128, KC, D], F8)
            wk8 = xw.tile([128, KC, KC, 128], F8)
            wq8 = xw.tile([128, KC, KC, 128], F8)
            for k2 in range(0, KC, 2):
                nc.sync.dma_start(wv8[:, k2:k2 + 2, :],
                                  io["wv8"].ap()[:, k2:k2 + 2, :])
                nc.sync.dma_start(x8s[:, 0, k2:k2 + 2, :],
                                  io["x8"].ap()[0][:, k2:k2 + 2, :])
            nci = 0
            for tb in range(4):
                if tb == 0:
                    pass
                else:
                    nc.sync.dma_start(x8s[:, tb], io["x8"].ap()[tb])
                if tb == 3:
                    nc.sync.dma_start(wk8[:], io["wk8"].ap())
                    nc.sync.dma_start(wq8[:], io["wq8"].ap())
                # V projection for this block: psum [128 tok, 512 dv]
                for tc_ in range(4 * tb, 4 * tb + 4):
                    for h in range(2):
                        ps = pskv.tile([128, 512], F32, name="ps_kv")
                        for k2 in range(0, KC, 2):
                            nc.tensor.matmul(
                                ps[:],
                                x8s[:, tb, k2:k2 + 2, bass.ts(tc_ % 4, 128)],
                                wv8[:, k2:k2 + 2, bass.ts(h, 512)],
                                start=(k2 == 0), stop=(k2 == KC - 2),
                                perf_mode=DR)
                        eng = copy_engines[nci % 2]; nci += 1
                        _scaled_copy(nc, eng, v8[:, tc_, bass.ts(h, 512)],
                                     ps[:], PS2Q)

            # ---- K projection (full rotated sequence) -> k8 ----
            nc.scalar.dma_start(msk_s[:], io["mask"].ap())
            nc.scalar.dma_start(x0b[:], io["x0b"].ap())
            nc.scalar.dma_start(br["wot0"][:], io["woutb"].ap()[:, 0])
            for tb in range(4):
                for m in range(KC):
                    ps = pskv.tile([128, TB], F32, name="ps_kv")
                    for k2 in range(0, KC, 2):
                        nc.tensor.matmul(
                            ps[:], wk8[:, k2:k2 + 2, m, :],
                            x8s[:, tb, k2:k2 + 2, :],
                            start=(k2 == 0), stop=(k2 == KC - 2),
                            perf_mode=DR)
                    eng = copy_engines[nci % 2]; nci += 1
                    _scaled_copy(nc, eng, k8[:, m, bass.ts(tb, TB)],
                                 ps[:], PS2Q)

            # ---- Q projection (own block = rotated block 0) -> q8 ----
            for m in range(KC):
                ps = pskv.tile([128, TB], F32, name="ps_kv")
                for k2 in range(0, KC, 2):
                    nc.tensor.matmul(
                        ps[:], wq8[:, k2:k2 + 2, m, :],
                        x8s[:, 0, k2:k2 + 2, :],
                        start=(k2 == 0), stop=(k2 == KC - 2), perf_mode=DR)
                eng = copy_engines[nci % 2]; nci += 1
                _scaled_copy(nc, eng, q8[:, m, :], ps[:], PS2Q)

        # ---------- attention (scores transposed: sT[tk, tq]) ----------
        with tc.tile_pool(name="attn", bufs=1) as ap_, \
             tc.tile_pool(name="stmp", bufs=6) as stp, \
             tc.tile_pool(name="ps_sc", bufs=4, space="PSUM") as ps_sc, \
             tc.tile_pool(name="ps_l", bufs=1, space="PSUM") as ps_lp:
            attn8 = ap_.tile([128, NTK, TB], F8)
            ones2 = ap_.tile([128, 2, 16], F8)   # 16B k-substep for dual-fp8
            nc.vector.memset(ones2[:], 1.0)
            ones_f = ap_.tile([1, 128], F32)
            nc.vector.memset(ones_f[:], 1.0)
            ones_r = ap_.tile([1, 128], F32R)
            nc.vector.tensor_copy(ones_r[:], ones_f[:])
            ebias = ap_.tile([128, 1], F32)
            nc.vector.memset(ebias[:], EXPBIAS)
            ps_l = ps_lp.tile([1, TB], F32)

            for tkc in range(NTK):
                ps = ps_sc.tile([128, TB], F32, name="ps_s")
                for k2 in range(0, KC, 2):
                    nc.tensor.matmul(
                        ps[:], k8[:, k2:k2 + 2, bass.ts(tkc, 128)],
                        q8[:, k2:k2 + 2, :],
                        start=(k2 == 0), stop=(k2 == KC - 2), perf_mode=DR)
                stmp = stp.tile([128, TB], BF16, name="stmp")
                nc.vector.tensor_tensor(out=stmp[:], in0=ps[:],
                                        in1=msk_s[:, tkc, :], op=OP.add)
                nc.scalar.activation(attn8[:, tkc, :], stmp[:], AF.Exp,
                                     bias=ebias[:], scale=EXPSCALE)
            # softmax normalizer: deferred ones-matmuls (2 chunks per mm)
            for tkc in range(1, NTK, 2):
                nc.tensor.matmul(ps_l[:], ones2[:, :, 0:1],
                                 attn8[:, tkc - 1:tkc + 1, :],
                                 start=(tkc == 1), stop=(tkc == NTK - 1),
                                 perf_mode=DR)

            # rs = 0.125/(sv*l) broadcast to all partitions via PE matmul
            rs_row = ap_.tile([1, TB], F32)
            nc.vector.reciprocal(rs_row[:], ps_l[:])
            rs_row2 = ap_.tile([1, TB], F32R)
            nc.vector.tensor_scalar_mul(rs_row2[:], rs_row[:], RSCONST)
            ps_b = ps_lp.tile([128, TB], F32, name="ps_b")
            nc.tensor.matmul(ps_b[:], ones_r[:], rs_row2[:],
                             start=True, stop=True)
            nc.vector.tensor_copy(rs_b[:], ps_b[:])

            # oT[dv, tq] = V.T @ attnT ; x1T = x0 + rs * oT
            for m in range(KC):
                ps = ps_sc.tile([128, TB], F32, name="ps_s")
                for t2 in range(0, NTK, 2):
                    nc.tensor.matmul(ps[:], v8[:, t2:t2 + 2, bass.ts(m, 128)],
                                     attn8[:, t2:t2 + 2, :],
                                     start=(t2 == 0), stop=(t2 == NTK - 2),
                                     perf_mode=DR)
                ot = stp.tile([128, TB], F32, name="otmp")
                nc.vector.tensor_tensor(out=ot[:], in0=ps[:], in1=rs_b[:],
                                        op=OP.mult)
                # adds alternate DVE/Pool so neither engine's serial chain
                # outlasts the AV matmul stream
                aeng = nc.vector if m % 2 else nc.gpsimd
                aeng.tensor_tensor(out=x1T[:, m, :], in0=ot[:],
                                   in1=x0b[:, m, :], op=OP.add)
                nc.scalar.activation(x18[:, m, :], x1T[:, m, :], AF.Copy,
                                     scale=SX1)

        # ---------- MLP ----------
        with tc.tile_pool(name="mlp", bufs=1) as mp, \
             tc.tile_pool(name="w1p", bufs=6) as w1p, \
             tc.tile_pool(name="w2p", bufs=6) as w2p, \
             tc.tile_pool(name="ps_h", bufs=6, space="PSUM") as ps_hp:
            b1_s = mp.tile([128, HC], F32)
            b2_s = mp.tile([128, KC], F32)
            nc.sync.dma_start(b1_s[:], io["b1t"].ap())
            nc.sync.dma_start(b2_s[:], io["b2t"].ap())
            hT = mp.tile([128, HC, TB], BF16)
            for m in range(HC):
                w1t = w1p.tile([128, KC, 128], F8, name="w1t")
                nc.sync.dma_start(w1t[:], io["w1b"].ap()[:, m])
                ps = ps_hp.tile([128, TB], F32, name="ps_mlp")
                for k2 in range(0, KC, 2):
                    nc.tensor.matmul(ps[:], w1t[:, k2:k2 + 2, :],
                                     x18[:, k2:k2 + 2, :],
                                     start=(k2 == 0), stop=(k2 == KC - 2),
                                     perf_mode=DR)
                nc.scalar.activation(hT[:, m, :], ps[:], AF.Gelu,
                                     bias=b1_s[:, m:m + 1], scale=G1SCALE)
            x2T = mp.tile([128, KC, TB], BF16)
            for m in range(KC):
                w2t = w2p.tile([128, HC, 128], BF16, name="w2t")
                nc.scalar.dma_start(w2t[:], io["w2b"].ap()[:, m])
                ps = ps_hp.tile([128, TB], F32, name="ps_mlp")
                for k in range(HC):
                    nc.tensor.matmul(ps[:], w2t[:, k, :], hT[:, k, :],
                                     start=(k == 0), stop=(k == HC - 1))
                # x2T = (psum + b2) + x1T
                nc.vector.scalar_tensor_tensor(
                    out=x2T[:, m, :], in0=ps[:], scalar=b2_s[:, m:m + 1],
                    in1=x1T[:, m, :], op0=OP.add, op1=OP.add)
                # split final-hidden AllGather: first half overlaps m=4..7
                if m == KC // 2 - 1:
                    nc.sync.dma_start(
                        bounce_x1[:].rearrange("(k p t) -> p k t",
                                               k=KC // 2, p=128),
                        x2T[:, :KC // 2, :])
                    if not _NO_COLL:
                        nc.gpsimd.collective_compute(
                            "AllGather", OP.bypass,
                            replica_groups=[list(range(N_CORES))],
                            ins=[bounce_x1.opt()], outs=[ag_x1.opt()])
            nc.sync.dma_start(
                bounce_x2[:].rearrange("(k p t) -> p k t", k=KC // 2, p=128),
                x2T[:, KC // 2:, :])
            if not _NO_COLL:
                nc.gpsimd.collective_compute(
                    "AllGather", OP.bypass,
                    replica_groups=[list(range(N_CORES))],
                    ins=[bounce_x2.opt()], outs=[ag_x2.opt()])
            # bridge hidden blocks (r=0,1): issued after every w2t load so a
            # collective-gated descriptor never blocks a sooner-needed one
            for r in range(2):
                nc.scalar.dma_start(
                    br["xf01"][:, KC * r:KC * r + KC // 2, :],
                    ag_x1[r].rearrange("(k p t) -> p k t",
                                       k=KC // 2, p=128))
            for r in range(2):
                nc.scalar.dma_start(
                    br["xf01"][:, KC * r + KC // 2:KC * (r + 1), :],
                    ag_x2[r].rearrange("(k p t) -> p k t",
                                       k=KC // 2, p=128))


def _logits(nc, tc, io, br, ag_x1, ag_x2):
    """Vocab-parallel logits over the AllGathered final hidden states."""
    out_d = io["logits"]
    copy_engines = [nc.vector, nc.scalar]
    xf01, wot0, outp = br["xf01"], br["wot0"], br["outp"]
    with tc.tile_pool(name="lgp", bufs=1) as lp, \
         tc.tile_pool(name="wop", bufs=2) as wop, \
         tc.tile_pool(name="ps_lg", bufs=8, space="PSUM") as ps_lg:
        # hidden blocks r=2..7 live in SBUF space recycled from the trunk
        xfr = lp.tile([128, (N_CORES - 2) * KC, TB], BF16)

        def xslice(r, k):
            if r < 2:
                return xf01[:, KC * r + k, :]
            return xfr[:, KC * (r - 2) + k, :]

        def load_xfr_half(r, half):
            ag = ag_x1 if half == 0 else ag_x2
            off = KC * (r - 2) + half * (KC // 2)
            nc.scalar.dma_start(
                xfr[:, off:off + KC // 2, :],
                ag[r].rearrange("(k p t) -> p k t", k=KC // 2, p=128))

        def mm_half(ps, r, t4, wot, half):
            for k in range(half * (KC // 2), (half + 1) * (KC // 2)):
                nc.tensor.matmul(
                    ps[:], xslice(r, k)[:, bass.ts(t4, 128)],
                    wot[:, k, :],
                    start=(k == 0), stop=(k == KC - 1))

        nci = 0

        def drain(ps, r, n, t4):
            nonlocal nci
            ot = outp.tile([128, VT], F32, name="og")
            eng = copy_engines[nci % 2]; nci += 1
            _scaled_copy(nc, eng, ot[:], ps[:], 1.0)
            nc.sync.dma_start(out_d.ap()[r, n, bass.ts(t4, 128), :], ot[:])

        # remaining hidden blocks (r=2..7): first-needed first
        for r in range(2, N_CORES):
            load_xfr_half(r, 0)
            load_xfr_half(r, 1)

        # bridge tiles: the first 8 output tiles start on the ag_x1 half of
        # the contraction so PE has work while ag_x2 is still in flight
        bridge = []
        for r in range(2):
            for t4 in range(4):
                ps = ps_lg.tile([128, VT], F32, name="ps_g")
                mm_half(ps, r, t4, wot0, 0)
                bridge.append((ps, r, t4))
        for ps, r, t4 in bridge:
            mm_half(ps, r, t4, wot0, 1)
            drain(ps, r, 0, t4)

        for n in range(8):
            if n == 0:
                wot = wot0
            else:
                wot = wop.tile([128, KC, VT], BF16, name="wot")
                nc.scalar.dma_start(wot[:], io["woutb"].ap()[:, n])
            for r in range(N_CORES):
                if n == 0 and r < 2:
                    continue   # bridge tiles already done
                for t4 in range(4):
                    ps = ps_lg.tile([128, VT], F32, name="ps_g")
                    mm_half(ps, r, t4, wot, 0)
                    mm_half(ps, r, t4, wot, 1)
                    drain(ps, r, n, t4)


def _build(repeat=1, phases="full"):
    nc = bacc.Bacc("TRN2", target_bir_lowering=False, debug=False,
                   num_devices=N_CORES)

    # ---- kernel I/O (per-core shards prepared on host) ----
    io = {}
    def inp(name, shape, dt=F32):
        io[name] = nc.dram_tensor(name, shape, dt, kind="ExternalInput")
    inp("x8", [4, 128, KC, TB], F8)
    inp("x0b", [128, KC, TB], BF16)
    inp("wq8", [128, KC, KC, 128], F8)
    inp("wk8", [128, KC, KC, 128], F8)
    inp("wv8", [128, KC, D], F8)
    inp("w1b", [128, HC, KC, 128], F8)
    inp("b1t", [128, HC])
    inp("w2b", [128, KC, HC, 128], BF16)
    inp("b2t", [128, KC])
    inp("woutb", [128, 8, KC, VT], BF16)
    inp("mask", [128, NTK, TB], BF16)
    io["logits"] = nc.dram_tensor("logits", [N_CORES, 8, TB, VT], F32,
                                  kind="ExternalOutput")

    with tile.TileContext(nc) as tc:
        with tc.tile_pool(name="dram", bufs=1, space="DRAM") as dp:
            for _ in range(repeat):  # repeat>1 is a timing-only variant
                bounce_x1 = dp.tile([XH_ELEMS], BF16, name="bounce_x1")
                bounce_x2 = dp.tile([XH_ELEMS], BF16, name="bounce_x2")
                ag_x1 = dp.tile([N_CORES, XH_ELEMS], BF16, name="ag_x1",
                                addr_space="Shared")
                ag_x2 = dp.tile([N_CORES, XH_ELEMS], BF16, name="ag_x2",
                                addr_space="Shared")
                with tc.tile_pool(name="bridge", bufs=1) as bp, \
                     tc.tile_pool(name="outp", bufs=6) as outp:
                    br = {
                        "xf01": bp.tile([128, 2 * KC, TB], BF16,
                                        name="xf01"),
                        "wot0": bp.tile([128, KC, VT], BF16, name="wot0"),
                        "outp": outp,
                    }
                    if phases in ("full", "trunk"):
                        _trunk(nc, tc, io, br, bounce_x1, bounce_x2,
                               ag_x1, ag_x2)
                    if phases in ("full", "logits"):
                        _logits(nc, tc, io, br, ag_x1, ag_x2)

    nc.compile()
    return nc


F8NP = ml_dtypes.float8_e4m3fn
BFNP = ml_dtypes.bfloat16


def _prep_shared(Wq, Wk, Wv, W1, b1, W2, b2, pos_emb):
    f = np.float32
    sh = {}
    sh["wq8"] = np.ascontiguousarray(
        (Wq * SW).reshape(KC, 128, KC, 128).transpose(1, 0, 2, 3)
    ).astype(F8NP)
    sh["wk8"] = np.ascontiguousarray(
        (Wk * SW).reshape(KC, 128, KC, 128).transpose(1, 0, 2, 3)
    ).astype(F8NP)
    sh["wv8"] = np.ascontiguousarray(
        (Wv * SW).reshape(KC, 128, D).transpose(1, 0, 2)).astype(F8NP)
    sh["w1b"] = np.ascontiguousarray(
        (W1 * SW1).reshape(KC, 128, HC, 128).transpose(1, 2, 0, 3)
    ).astype(F8NP)
    sh["b1t"] = np.ascontiguousarray(b1.reshape(HC, 128).T, dtype=f)
    sh["w2b"] = np.ascontiguousarray(
        W2.reshape(HC, 128, KC, 128).transpose(1, 2, 0, 3)).astype(BFNP)
    sh["b2t"] = np.ascontiguousarray(b2.reshape(KC, 128).T, dtype=f)

    # per-j rotated block order and causal masks.
    # rotation: the core owning block j sees blocks in order [j, j+1, j+2,
    # j+3] (mod 4), so its own 512 tokens are always columns 0:TB.
    orders = [[(j + i) % 4 for i in range(4)] for j in range(4)]
    masks = []
    rr = np.arange(128)[:, None]
    cc = np.arange(TB)[None, :]
    for j in range(4):
        m = np.empty((NTK, 128, TB), dtype=f)
        for tkc in range(NTK):
            gtk = TB * orders[j][tkc // 4] + 128 * (tkc % 4) + rr
            m[tkc] = np.where(gtk <= TB * j + cc, 0.0, MASKVAL)
        masks.append(
            np.ascontiguousarray(m.transpose(1, 0, 2)).astype(BFNP))
    return sh, orders, masks


def make_in_maps(idx, tok_emb, pos_emb, Wq, Wk, Wv, W1, b1, W2, b2,
                 Wout, bout):
    f = np.float32
    tok_emb = np.asarray(tok_emb, dtype=f)
    pos = np.asarray(pos_emb, dtype=f)[:T]
    idx = np.asarray(idx)
    sh, orders, masks = _prep_shared(
        np.asarray(Wq, f), np.asarray(Wk, f), np.asarray(Wv, f),
        np.asarray(W1, f), np.asarray(b1, f), np.asarray(W2, f),
        np.asarray(b2, f), pos)
    Wout = np.asarray(Wout, f)

    x_full = [tok_emb[np.asarray(idx[b], dtype=np.int64)] + pos
              for b in range(B)]
    in_maps = []
    for c in range(N_CORES):
        b, j = c // 4, c % 4
        xr = np.concatenate([x_full[b][TB * br:TB * (br + 1)]
                             for br in orders[j]])           # [T, D] rotated
        x8pm = (xr.T * SX).reshape(KC, 128, T).transpose(1, 0, 2)  # [128,KC,T]
        m = dict(sh)
        m["x8"] = np.ascontiguousarray(
            x8pm.reshape(128, KC, 4, TB).transpose(2, 0, 1, 3)).astype(F8NP)
        m["x0b"] = np.ascontiguousarray(
            x_full[b][TB * j:TB * (j + 1)].T.reshape(KC, 128, TB)
            .transpose(1, 0, 2)).astype(BFNP)
        m["mask"] = masks[j]
        ws = Wout[:, VS * c:VS * (c + 1)]
        m["woutb"] = np.ascontiguousarray(
            ws.reshape(KC, 128, 8, VT).transpose(1, 2, 0, 3)).astype(BFNP)
        in_maps.append(m)
    return in_maps


def kernel(idx, tok_emb, pos_emb, Wq, Wk, Wv, W1, b1, W2, b2, Wout, bout):
    if "nc" not in _STATE:
        _STATE["nc"] = _build()
    nc = _STATE["nc"]

    in_maps = make_in_maps(idx, tok_emb, pos_emb, Wq, Wk, Wv, W1, b1, W2,
                           b2, Wout, bout)
    res = bass_utils.run_bass_kernel_spmd(nc, in_maps,
                                          core_ids=list(range(N_CORES)))
    _STATE["last_results"] = res

    out = np.empty((B * T, V), dtype=np.float32)
    for c in range(N_CORES):
        lg = res.results[c]["logits"]             # [8, 8, 512, 500]
        out[:, VS * c:VS * (c + 1)] = (
            lg.transpose(0, 2, 1, 3).reshape(B * T, VS))
    out += np.asarray(bout, np.float32)[None, :]
    return out.reshape(B, T, V)
